# revision 16
# baseline (speedup 1.0000x reference)
"""Trainium2 Bass kernel for nn_LocalgridAttention2d (7x7 local window
attention, B=16 H=W=56 D=768 nh=12).

Sharding: data-parallel over batch, 2 images per core across 8 cores.

Per-core pipeline (all matmuls bf16 x bf16 -> fp32 psum):
  Phase 1 (QKV): x^T via DMA-xbar transpose; Q^T,K^T computed
    feature-major [128=(2 heads x 64d), tokens]; V token-major.
    K^T and V written zero-padded (64x64 padded grid) to DRAM scratch.
  Phase 2 (attention): 8x8 query tiles, 16x16 key patches (256 keys),
    two heads packed per PE pass via 2D tile_position. Softmax uses the
    multiplicative-bias trick: E = exp(s) * EB where EB = exp(bias+mask)
    is a host-precomputed table (masked entries exactly 0). A^T obtained
    with DMA-xbar transposes; AV contracts over 128-key chunks against
    V "strips" (vertical 16-wide bands, 128 pixels per block).
  Phase 3 (proj): ao^T via DMA-xbar transpose, token-major matmul + bias.
"""
import sys
import numpy as np
import ml_dtypes

for _p in ("/opt/trn_rl_repo",):
    if _p not in sys.path:
        sys.path.insert(0, _p)

import concourse.bass as bass
import concourse.mybir as mybir
import concourse.tile as tile
from concourse import bacc
from concourse import bass_utils

F32 = mybir.dt.float32
BF16 = mybir.dt.bfloat16
AF = mybir.ActivationFunctionType
ALU = mybir.AluOpType

B, H, W, D = 16, 56, 56, 768
NH, HD = 12, 64
KS, P = 7, 3
PG = 64          # padded grid rows/cols (image pixel (r,c) -> (r+3, c+3))
NT = 7           # 8x8 query tiles per axis
NC_ = 8          # cores
BI = B // NC_    # images per core (2)
TOK = H * W      # 3136 tokens per image
NG = 7           # phase-1 token groups per image (8 rows = 448 tokens)
GT = TOK // NG   # 448
NHP = NH // 2    # 6 head pairs

bf16 = ml_dtypes.bfloat16


# ---------------------------------------------------------------- host prep
def _build_eb_tables(rel_bias):
    """EB[hp, tt, 128=(2h,8qi,8qj), 256=(16ki,16kj)] = exp(bias) or 0."""
    rb = np.asarray(rel_bias, np.float32).reshape(NH, KS, KS)
    qi = np.arange(8)[:, None, None, None]
    ki = np.arange(16)[None, None, :, None]
    kj = np.arange(16)[None, None, None, :]
    qj = np.arange(8)[None, :, None, None]
    di, dj = ki - qi, kj - qj
    win = (di >= 0) & (di <= 6) & (dj >= 0) & (dj <= 6)
    dic, djc = np.clip(di, 0, 6), np.clip(dj, 0, 6)
    tables = np.zeros((NHP, 9, 128, 256), np.float32)
    rep = {0: 0, 1: 3, 2: 6}
    for rc in range(3):
        for cc in range(3):
            ti, tj = rep[rc], rep[cc]
            kr = 8 * ti + ki - P + 0 * qi
            kc = 8 * tj + kj - P + 0 * qi
            valid = win & (kr >= 0) & (kr < H) & (kc >= 0) & (kc < W)
            for h in range(NH):
                ebt = np.where(valid, np.exp(rb[h])[dic, djc], 0.0)
                tables[h // 2, rc * 3 + cc,
                       (h % 2) * 64:(h % 2) * 64 + 64] = ebt.reshape(64, 256)
    return tables.astype(bf16)


def _tile_type(ti, tj):
    rc = 0 if ti == 0 else (2 if ti == NT - 1 else 1)
    cc = 0 if tj == 0 else (2 if tj == NT - 1 else 1)
    return rc * 3 + cc


# ------------------------------------------------------------- kernel build
def _emit(tc):
    nc = tc.nc
    x_d = nc.dram_tensor("x_bf", [BI * TOK, D], BF16, kind="ExternalInput")
    wT_d = nc.dram_tensor("wT", [D, 3 * D], BF16, kind="ExternalInput")
    qkb_d = nc.dram_tensor("qkb", [128, 12], F32, kind="ExternalInput")
    vb_d = nc.dram_tensor("vb", [128, D], F32, kind="ExternalInput")
    pwT_d = nc.dram_tensor("pwT", [D, D], BF16, kind="ExternalInput")
    pb_d = nc.dram_tensor("pb", [128, D], F32, kind="ExternalInput")
    eb_d = nc.dram_tensor("eb", [NHP, 9, 128, 256], BF16, kind="ExternalInput")
    out_d = nc.dram_tensor("out", [BI * TOK, D], F32, kind="ExternalOutput")

    with tc.tile_pool(name="dram", bufs=1, space="DRAM") as dpool:
        qT_s = dpool.tile([BI, NHP, 128, TOK], BF16)
        kT_s = dpool.tile([BI, NHP, 128, PG * PG], BF16)
        v_s = dpool.tile([BI, PG, PG, D], BF16)
        ao_s = dpool.tile([BI, TOK, D], BF16)

        with tc.tile_pool(name="const", bufs=1) as cpool:
            wT_sb = cpool.tile([128, 6 * 3 * D], BF16)     # [:, dc*2304+f]
            for dc in range(6):
                nc.sync.dma_start(wT_sb[:, dc * 3 * D:(dc + 1) * 3 * D],
                                  wT_d[dc * 128:(dc + 1) * 128, :])
            qkb_sb = cpool.tile([128, 12], F32)
            nc.sync.dma_start(qkb_sb[:], qkb_d[:])
            vb_sb = cpool.tile([128, D], F32)
            nc.sync.dma_start(vb_sb[:], vb_d[:])
            pwT_sb = cpool.tile([128, 6 * D], BF16)
            for dc in range(6):
                nc.sync.dma_start(pwT_sb[:, dc * D:(dc + 1) * D],
                                  pwT_d[dc * 128:(dc + 1) * 128, :])
            pb_sb = cpool.tile([128, D], F32)
            nc.sync.dma_start(pb_sb[:], pb_d[:])
            zero_sb = cpool.tile([128, 3840], BF16)
            nc.vector.memset(zero_sb[:], 0.0)

            _zero_borders(nc, zero_sb, kT_s, v_s)
            _phase1(tc, nc, x_d, wT_sb, qkb_sb, vb_sb, qT_s, kT_s, v_s)
            _phase2(tc, nc, qT_s, kT_s, v_s, eb_d, ao_s)
            _phase3(tc, nc, ao_s, pwT_sb, pb_sb, out_d)
    return out_d


def _zero_borders(nc, zero_sb, kT_s, v_s):
    kT = kT_s.rearrange("b n p (r c) -> b n p r c", r=PG)
    for img in range(BI):
        for fc in range(NHP):
            t = kT[img, fc]
            nc.sync.dma_start(t[:, 0:3, :], zero_sb[:, :192])
            nc.sync.dma_start(t[:, 59:64, :], zero_sb[:, :320])
            nc.sync.dma_start(t[:, 3:59, 0:3], zero_sb[:, :168])
            nc.sync.dma_start(t[:, 3:59, 59:64], zero_sb[:, :280])
        v = v_s[img]
        nc.sync.dma_start(v[0:3], zero_sb[:, :1152])
        nc.sync.dma_start(v[59:64], zero_sb[:, :1920])
        nc.sync.dma_start(v[3:59, 0:3], zero_sb[0:56, :2304])
        nc.sync.dma_start(v[3:59, 59:64], zero_sb[0:56, :3840])


def _phase1(tc, nc, x_d, wT_sb, qkb_sb, vb_sb, qT_s, kT_s, v_s):
    """QKV projection. Per image, 7 groups of 8 pixel rows (448 tokens)."""
    kT = kT_s.rearrange("b n p (r c) -> b n p r c", r=PG)
    with (
        tc.tile_pool(name="p1", bufs=2) as pool,
        tc.tile_pool(name="p1qk", bufs=4) as qkpool,
        tc.tile_pool(name="p1ps", bufs=4, space="PSUM") as pspool,
        tc.tile_pool(name="p1psv", bufs=4, space="PSUM") as psvpool,
    ):
        for img in range(BI):
            for g in range(NG):
                rows = slice(img * TOK + g * GT, img * TOK + (g + 1) * GT)
                xT_sb = pool.tile([128, 6 * GT], BF16, tag="xt")
                for dc in range(6):
                    nc.sync.dma_start(
                        out=xT_sb[:, dc * GT:(dc + 1) * GT],
                        in_=x_d[rows, dc * 128:(dc + 1) * 128],
                        transpose=True)
                # Q^T and K^T: feature-major [128, 448]
                for fc in range(12):
                    ps = pspool.tile([128, GT], F32, tag="qk")
                    for dc in range(6):
                        nc.tensor.matmul(
                            ps[:],
                            lhsT=wT_sb[:, dc * 3 * D + fc * 128:
                                       dc * 3 * D + (fc + 1) * 128],
                            rhs=xT_sb[:, dc * GT:(dc + 1) * GT],
                            start=(dc == 0), stop=(dc == 5))
                    qk_sb = qkpool.tile([128, GT], BF16, tag="qk")
                    if fc < 6:
                        # write Q tile-major: dst free = (tj, qi, qj) while
                        # psum src is (qi, tj, qj) — permute in the evac op
                        out_ap = qk_sb[:].rearrange(
                            "p (t a b) -> p a t b", a=8, t=NT)
                        in_ap = ps[:].rearrange(
                            "p (a t b) -> p a t b", a=8, t=NT)
                    else:
                        out_ap = qk_sb[:]
                        in_ap = ps[:]
                    if fc % 2 == 0:
                        nc.scalar.activation(out_ap, in_ap, AF.Identity,
                                             bias=qkb_sb[:, fc:fc + 1])
                    else:
                        nc.vector.tensor_scalar_add(out_ap, in_ap,
                                                    qkb_sb[:, fc:fc + 1])
                    if fc < 6:
                        nc.sync.dma_start(
                            qT_s[img, fc, :, g * GT:(g + 1) * GT], qk_sb[:])
                    else:
                        nc.sync.dma_start(
                            kT[img, fc - 6, :, 8 * g + 3:8 * g + 11, 3:59],
                            qk_sb[:].rearrange("p (r c) -> p r c", r=8))
                # V: token-major, 4 sub-chunks of 112 tokens (2 pixel rows)
                for sc in range(4):
                    vv_sb = qkpool.tile([112, D], BF16, tag="vv")
                    for fh in range(2):
                        psv = psvpool.tile([112, 384], F32, tag="v")
                        for dc in range(6):
                            nc.tensor.matmul(
                                psv[:],
                                lhsT=xT_sb[:, dc * GT + sc * 112:
                                           dc * GT + (sc + 1) * 112],
                                rhs=wT_sb[:, dc * 3 * D + 2 * D + fh * 384:
                                          dc * 3 * D + 2 * D + (fh + 1) * 384],
                                start=(dc == 0), stop=(dc == 5))
                        nc.vector.scalar_tensor_tensor(
                            vv_sb[:, fh * 384:(fh + 1) * 384], psv[:], 1.0,
                            vb_sb[:112, fh * 384:(fh + 1) * 384],
                            op0=ALU.mult, op1=ALU.add)
                    r0 = 8 * g + 2 * sc + 3
                    nc.sync.dma_start(
                        v_s[img, r0:r0 + 2, 3:59, :], vv_sb[:])


def _phase2(tc, nc, qT_s, kT_s, v_s, eb_d, ao_s):
    """Local attention. Per (image, head-pair): 49 tiles."""
    ao = ao_s.rearrange("b (r c) f -> b r c f", r=H)
    with (
        tc.tile_pool(name="p2in", bufs=2) as inpool,
        tc.tile_pool(name="p2w", bufs=3) as wpool,
        tc.tile_pool(name="p2ps", bufs=3, space="PSUM") as pspool,
        tc.tile_pool(name="p2psa", bufs=3, space="PSUM") as psapool,
    ):
        for img in range(BI):
            for hp in range(NHP):
                qT2 = inpool.tile([128, TOK], BF16, tag="q2")
                nc.sync.dma_start(qT2[:], qT_s[img, hp])
                kgrid = kT_s[img, hp].rearrange("p (r c) -> p r c", r=PG)
                kstrips = []
                for tj in range(NT):
                    ks = inpool.tile([128, PG * 16], BF16, tag=f"ks{tj}")
                    nc.sync.dma_start(
                        ks[:].rearrange("p (r c) -> p r c", c=16),
                        kgrid[:, :, 8 * tj:8 * tj + 16])
                    kstrips.append(ks)
                eb2 = inpool.tile([128, 9 * 256], BF16, tag="eb2")
                nc.sync.dma_start(
                    eb2[:].rearrange("p (t k) -> p t k", t=9),
                    eb_d[hp].rearrange("t p k -> p t k"))
                strips = []
                for tj in range(NT):
                    st = inpool.tile([128, 8 * 128], BF16, tag=f"st{tj}")
                    for b in range(8):
                        nc.sync.dma_start(
                            st[:, b * 128:(b + 1) * 128],
                            v_s[img, 8 * b:8 * b + 8, 8 * tj:8 * tj + 16,
                                hp * 128:(hp + 1) * 128])
                    strips.append(st)
                for ti in range(NT):
                    for tj in range(NT):
                        tt = _tile_type(ti, tj)
                        toff = (ti * NT + tj) * 64
                        s_ps = pspool.tile([128, 256], F32, tag="s")
                        for hl in range(2):
                            nc.tensor.matmul(
                                s_ps[64 * hl:64 * hl + 64, :],
                                lhsT=qT2[64 * hl:64 * hl + 64,
                                         toff:toff + 64],
                                rhs=kstrips[tj][64 * hl:64 * hl + 64,
                                                ti * 128:ti * 128 + 256],
                                start=True, stop=True,
                                skip_group_check=(hl == 1),
                                tile_position=(64 * hl, 64 * hl))
                        t_sb = wpool.tile([128, 256], BF16, tag="t")
                        nc.scalar.activation(t_sb[:], s_ps[:], AF.Exp)
                        e_sb = wpool.tile([128, 256], BF16, tag="e")
                        r_sb = wpool.tile([128, 1], F32, tag="r")
                        nc.vector.scalar_tensor_tensor(
                            e_sb[:], t_sb[:], 1.0,
                            eb2[:, tt * 256:(tt + 1) * 256],
                            op0=ALU.mult, op1=ALU.mult, accum_out=r_sb[:])
                        eT_sb = wpool.tile([128, 2, 128], BF16, tag="eT")
                        for c in range(2):
                            nc.sync.dma_start(
                                out=eT_sb[:, c, :],
                                in_=e_sb[:, c * 128:(c + 1) * 128],
                                transpose=True)
                        ao_ps = psapool.tile([128, HD], F32, tag="ao")
                        for c in range(2):
                            for hl in range(2):
                                nc.tensor.matmul(
                                    ao_ps[64 * hl:64 * hl + 64, :],
                                    lhsT=eT_sb[:, c, 64 * hl:64 * hl + 64],
                                    rhs=strips[tj][:, (ti + c) * 128 + 64 * hl:
                                                   (ti + c) * 128 + 64 * hl + 64],
                                    start=(c == 0), stop=(c == 1),
                                    skip_group_check=(hl == 1),
                                    tile_position=(0, 64 * hl))
                        rinv = wpool.tile([128, 1], F32, tag="rinv")
                        nc.vector.reciprocal(rinv[:], r_sb[:])
                        ao_sb = wpool.tile([128, HD], BF16, tag="aosb")
                        nc.vector.tensor_scalar_mul(ao_sb[:], ao_ps[:],
                                                    rinv[:])
                        for hl in range(2):
                            nc.sync.dma_start(
                                ao[img, 8 * ti:8 * ti + 8, 8 * tj:8 * tj + 8,
                                   hp * 128 + 64 * hl:hp * 128 + 64 * hl + 64],
                                ao_sb[64 * hl:64 * hl + 64, :])


def _phase3(tc, nc, ao_s, pwT_sb, pb_sb, out_d):
    """Output projection, token-major."""
    NCH = 25  # 24x128 + 64 token chunks per image
    with (
        tc.tile_pool(name="p3", bufs=3) as pool,
        tc.tile_pool(name="p3ps", bufs=4, space="PSUM") as pspool,
    ):
        for img in range(BI):
            for ch in range(NCH):
                cs = 128 if ch < 24 else 64
                t0 = ch * 128
                aoT_sb = pool.tile([128, 6, 128], BF16, tag="aoT")
                for dc in range(6):
                    nc.sync.dma_start(
                        out=aoT_sb[:, dc, :cs],
                        in_=ao_s[img, t0:t0 + cs, dc * 128:(dc + 1) * 128],
                        transpose=True)
                out_sb = pool.tile([128, D], F32, tag="po")
                for fh in range(2):
                    ps = pspool.tile([128, 384], F32, tag="o")
                    for dc in range(6):
                        nc.tensor.matmul(
                            ps[:cs, :],
                            lhsT=aoT_sb[:, dc, :cs],
                            rhs=pwT_sb[:, dc * D + fh * 384:
                                       dc * D + (fh + 1) * 384],
                            start=(dc == 0), stop=(dc == 5))
                    nc.vector.scalar_tensor_tensor(
                        out_sb[:cs, fh * 384:(fh + 1) * 384], ps[:cs, :], 1.0,
                        pb_sb[:cs, fh * 384:(fh + 1) * 384],
                        op0=ALU.mult, op1=ALU.add)
                nc.sync.dma_start(
                    out_d[img * TOK + t0:img * TOK + t0 + cs, :],
                    out_sb[:cs, :])


_CACHE = {}


def _get_compiled():
    if "nc" not in _CACHE:
        nc = bacc.Bacc("TRN2", target_bir_lowering=False, debug=False,
                       num_devices=NC_)
        with tile.TileContext(nc) as tc:
            _emit(tc)
        nc.compile()
        _CACHE["nc"] = nc
    return _CACHE["nc"]


def _make_in_maps(inputs):
    return _prep_inputs(**inputs)


def _prep_inputs(x, qkv_w, qkv_b, rel_bias, proj_w, proj_b):
    x = np.asarray(x, np.float32)
    qkv_w = np.asarray(qkv_w, np.float32)
    qkv_b = np.asarray(qkv_b, np.float32)
    rel_bias = np.asarray(rel_bias, np.float32)
    proj_w = np.asarray(proj_w, np.float32)
    proj_b = np.asarray(proj_b, np.float32)

    scale = 1.0 / np.sqrt(HD)
    wq = qkv_w.copy()
    wq[:D] *= scale
    qb = qkv_b.copy()
    qb[:D] *= scale

    wT = np.ascontiguousarray(wq.T).astype(bf16)           # [d, 3D]
    qkb = np.ascontiguousarray(
        qb[:2 * D].reshape(12, 128).T).astype(np.float32)  # [128, 12]
    vb = np.broadcast_to(qb[2 * D:], (128, D)).copy().astype(np.float32)
    pwT = np.ascontiguousarray(proj_w.T).astype(bf16)
    pb = np.broadcast_to(proj_b, (128, D)).copy().astype(np.float32)
    eb = _build_eb_tables(rel_bias)
    x_bf = x.reshape(B, TOK, D).astype(bf16)

    in_maps = []
    for core in range(NC_):
        xs = np.ascontiguousarray(
            x_bf[core * BI:(core + 1) * BI].reshape(BI * TOK, D))
        in_maps.append({"x_bf": xs, "wT": wT, "qkb": qkb, "vb": vb,
                       "pwT": pwT, "pb": pb, "eb": eb})
    return in_maps


def kernel(x, qkv_w, qkv_b, rel_bias, proj_w, proj_b):
    nc = _get_compiled()
    in_maps = _prep_inputs(x, qkv_w, qkv_b, rel_bias, proj_w, proj_b)
    res = bass_utils.run_bass_kernel_spmd(nc, in_maps,
                                          core_ids=list(range(NC_)))
    out = np.concatenate([r["out"] for r in res.results], axis=0)
    return out.reshape(B, H, W, D)
